# revision 52
# baseline (speedup 1.0000x reference)
"""Two-layer GAT on 8 Trainium2 NeuronCores (Bass/Tile) — v3.

Key changes vs v2:
  - Unified edge schedule: both layers share idx/dl/sT streams. Node table
    rows are laid out chunk-major (row = k*10240 + c*1280 + rr, K=5 chunks)
    so the layer-2 AllGather can be issued in 5 chunks overlapping the L1
    tail, and both layers gather with identical ids.
  - Merged gathers: one dma_gather per half per 2-window group (vs 4 per
    window) -> ~8x fewer Q7 fixed costs.
  - Edge-major one-hot s1 built on-device (DVE is_equal vs shipped dl
    stream) instead of streamed from host (-29MB/core DMA). Dst-major sT
    still streamed.
  - Phase-0 overlap: T1 split into A/B tensors at the gather-half boundary;
    L1 A-half gathers start when T1A (40%) is written.
  - Per-window dst a_dst-folds come from small on-device matmuls (x_own for
    L1, flush1 PSUM for L2) -> no strided per-window fold loads.

Table row layouts (bf16):
  T1 [51200, 384]: 0:256 feats | 256:260 ones | 260:264 a_src fold | pad.
  T2g [51200, 256]: 0:160 feats | 160:164 ones | 164:168 a_src fold | pad.
"""
import math
import os
import sys
import types

sys.path.insert(0, "/opt/trn_rl_repo")

import numpy as np
import ml_dtypes

BF16 = ml_dtypes.bfloat16
FP8 = ml_dtypes.float8_e4m3fn

FULL_CFG = dict(
    N=50000, E=800000, F_IN=128, HID=64, HEADS=4, NCLS=40, NEG=0.2,
    CORES=8, QUEUES=4,
)
WIN = 128
PADLOC = 200.0
G = 2               # windows per gather group
W = 49              # dst windows per core (6250/128)
NPC = 6250
# three gather thirds: windows 0-19 / 20-39 / 40-48 (padded to 1280 rows)
RPC = (2560, 2560, 1280)          # rows per core per third
RBASE = (0, 20480, 40960)         # table row base of each third
WLIM = (0, 20, 40, 49)            # window ranges of the thirds
ROWS = 51200


def _wrap_idx(idx):
    """int16 gather-index layout: index j at [j%16, j//16], replicated to all
    eight 16-partition groups."""
    idx = np.asarray(idx, dtype=np.int16)
    n16 = max((len(idx) + 15) // 16, 1)
    arr = np.zeros((128, n16), dtype=np.int16)
    w = np.zeros(n16 * 16, dtype=np.int16)
    w[:len(idx)] = idx
    w = w.reshape(-1, 16).T
    for g in range(8):
        arr[g * 16:(g + 1) * 16, :] = w
    return arr


def _remap(node):
    """Global node id -> third-major table row (matches the 3-stage
    AllGather layout: thirds in order, core-major within each)."""
    c = node // NPC
    r = node % NPC
    return np.where(
        r < 2560, c * 2560 + r,
        np.where(r < 5120, 20480 + c * 2560 + (r - 2560),
                 40960 + c * 1280 + (r - 5120)))


def build_schedules(src, dst, cfg):
    """Host-side edge schedule, shared by both layers.

    Slot order: group-major; within a group all A-half chunks (w-major),
    then all B-half chunks. Chunk counts CH[w, half] maxed over cores so
    the SPMD program is shape-uniform."""
    C = cfg["CORES"]
    g_all = _remap(src)
    dc = dst // NPC
    e_r = dst % NPC

    groups = [list(range(gr * G, min((gr + 1) * G, W)))
              for gr in range((W + G - 1) // G)]

    NH = 3
    cores = []
    cnt = np.zeros((C, W, NH), dtype=np.int64)
    for c in range(C):
        m = dc == c
        g, r = g_all[m], e_r[m]
        w = r // WIN
        half = np.where(g < RBASE[1], 0, np.where(g < RBASE[2], 1, 2))
        np.add.at(cnt[c], (w, half), 1)
        cores.append((g, r, w, half))
    CH = (np.max(cnt, axis=0) + 127) // 128          # [W, NH]

    # global chunk order: per group, thirds in order, w-major within each
    chunk_w, chunk_h = [], []
    for wins in groups:
        for h in range(NH):
            for w in wins:
                chunk_w += [w] * int(CH[w, h])
                chunk_h += [h] * int(CH[w, h])
    chunk_w = np.array(chunk_w)
    chunk_h = np.array(chunk_h)
    NCtot = len(chunk_w)
    # first slot of each (w, h) segment in the global slot space
    chunk_base = np.zeros((W, NH), np.int64)
    seen = set()
    for ci in range(NCtot):
        key = (int(chunk_w[ci]), int(chunk_h[ci]))
        if key not in seen:
            seen.add(key)
            chunk_base[key] = ci * 128

    slot_h = np.repeat(chunk_h, 128)
    tot = NCtot * 128
    rbase = np.array(RBASE)

    per_core = []
    for c in range(C):
        g, r, w, half = cores[c]
        # stable sort by (group, third, window); then slot position within
        # each (w, h) segment
        sortkey = ((w // G) * NH + half) * W + w
        order = np.argsort(sortkey, kind="stable")
        sw, sh, sg, sr = w[order], half[order], g[order], r[order]
        counts = cnt[c]                               # [W, NH]
        seg_start = np.zeros((W, NH), np.int64)
        posn = 0
        for wins in groups:
            for h in range(NH):
                for w_ in wins:
                    seg_start[w_, h] = posn
                    posn += int(counts[w_, h])
        within = np.arange(len(sw)) - seg_start[sw, sh]
        dest = chunk_base[sw, sh] + within
        assert (within < CH[sw, sh] * 128).all(), "chunk overflow"
        assert (within >= 0).all()

        gidx = np.zeros(tot, np.int64)
        gidx[dest] = sg - rbase[sh]
        dlpad = np.full(tot, PADLOC, np.float32)
        dlpad[dest] = sr % WIN

        data = {}
        for h in range(NH):
            data[f"idx{h}"] = _wrap_idx(gidx[slot_h == h])
        valid = dlpad < 128
        vi = np.nonzero(valid)[0]
        dv = dlpad[valid].astype(np.int64)
        sT = np.zeros((128, tot), dtype=FP8)
        sT[dv, vi] = 1
        data["sT"] = sT
        s1 = np.zeros((128, tot), dtype=FP8)
        s1[vi % 128, (vi // 128) * 128 + dv] = 1
        data["s1"] = s1
        per_core.append(data)

    # per-group chunk slices for the device loop
    ginfo = []
    ci0 = 0
    for wins in groups:
        cH = [int(sum(CH[w, h] for w in wins)) for h in range(NH)]
        hoff = np.concatenate([[0], np.cumsum(cH)])
        wmeta = []
        pos_h = list(hoff[:NH])
        for w in wins:
            cl = []
            for h in range(NH):
                cl.append((int(pos_h[h]), int(CH[w, h])))
                pos_h[h] += int(CH[w, h])
            wmeta.append((w, cl))
        ginfo.append(dict(cH=cH, ci0=ci0, wins=wmeta))
        ci0 += int(hoff[NH])
    Ls = [int(np.sum(slot_h == h)) for h in range(NH)]
    common = dict(CH=CH, groups=groups, ginfo=ginfo, NCtot=NCtot, Ls=Ls)
    return common, per_core


def fold_params(W1, a_src1, a_dst1, W2, a_src2, a_dst2, cfg):
    """W*_ext column layout: [features (head-major) | as-fold | ad-fold]."""
    H, HID, NCLS = cfg["HEADS"], cfg["HID"], cfg["NCLS"]
    f1 = [W1[:, h * HID:(h + 1) * HID] @ a_src1[h] for h in range(H)]
    f2 = [W1[:, h * HID:(h + 1) * HID] @ a_dst1[h] for h in range(H)]
    W1_ext = np.concatenate([W1] + [np.stack(f1, 1), np.stack(f2, 1)], axis=1)
    g1 = [W2[:, h * NCLS:(h + 1) * NCLS] @ a_src2[h] for h in range(H)]
    g2 = [W2[:, h * NCLS:(h + 1) * NCLS] @ a_dst2[h] for h in range(H)]
    W2_ext = np.concatenate([W2] + [np.stack(g1, 1), np.stack(g2, 1)], axis=1)
    return np.ascontiguousarray(W1_ext), np.ascontiguousarray(W2_ext)


# ---------------------------------------------------------------------------
# device program
# ---------------------------------------------------------------------------

def build_program(cfg, common, sim_queues=False):
    import concourse.bacc as bacc
    import concourse.bass as bass
    import concourse.mybir as mybir
    import concourse.tile as tile
    from concourse.tile import ScopedClock

    def _drain_patch(self, tick_clock, wait_clock):
        nc_ = self.nc
        probe = nc_.sync.nop(nofuse=True)
        wait_clock.add_sem_waits(probe.ins,
                                 ScopedClock({None: tick_clock.global_clock}))
        si = probe.ins.sync_info
        if si is not None and si.on_wait is not None and len(si.on_wait) > 1:
            extra = list(si.on_wait[1:])
            si.on_wait = si.on_wait[:1]
            for wt in extra:
                n2 = nc_.sync.nop(nofuse=True)
                si2 = n2.ins.sync_info
                if si2 is None:
                    n2.ins.sync_info = mybir.SyncInfo(on_wait=[wt], on_update=[])
                else:
                    lst = si2.on_wait or []
                    lst.append(wt)
                    si2.on_wait = lst
        nc_.sync.drain()
        nc_.all_engine_barrier()
        popped = nc_._tile_sem_poison_stack.pop()
        assert popped is self._sem_poison
        nc_.clear_and_free_semaphores(list(self.sems.allocated().values()))
        nc_.all_engine_barrier()
    tile.TileContext._drain_and_barrier = _drain_patch

    C = cfg["CORES"]
    H, HID, NCLS, F_IN = cfg["HEADS"], cfg["HID"], cfg["NCLS"], cfg["F_IN"]
    NEG = cfg["NEG"]
    CH, ginfo = common["CH"], common["ginfo"]
    NCtot, Ls = common["NCtot"], common["Ls"]
    F1, F2 = H * HID, H * NCLS                       # 256 / 160
    T1W, T2W = 512, 256      # T1 rows are fp8 (512B); T2 rows bf16 (512B)
    bf16, f32, i16 = mybir.dt.bfloat16, mybir.dt.float32, mybir.dt.int16
    fp8 = mybir.dt.float8e4
    AF = mybir.ActivationFunctionType
    OP = mybir.AluOpType

    nc = bacc.Bacc("TRN2", target_bir_lowering=False, debug=False,
                   num_swdge_queues=cfg["QUEUES"])

    # ---- I/O ----
    xT = nc.dram_tensor("xT", [F_IN, ROWS], bf16, kind="ExternalInput")
    xown = nc.dram_tensor("xown", [F_IN, W * 128], bf16, kind="ExternalInput")
    w1e = nc.dram_tensor("w1e", [F_IN, F1 + 2 * H], bf16, kind="ExternalInput")
    w2e = nc.dram_tensor("w2e", [HID, F2 + 2 * H], bf16, kind="ExternalInput")
    b1t = nc.dram_tensor("b1t", [128, HID], f32, kind="ExternalInput")
    b2t = nc.dram_tensor("b2t", [128, NCLS], f32, kind="ExternalInput")
    ident_d = nc.dram_tensor("ident", [128, 128], f32, kind="ExternalInput")
    idx_d = [nc.dram_tensor(f"idx{h}", [128, max(Ls[h] // 16, 1)], i16,
                            kind="ExternalInput") for h in range(3)]
    sT_d = nc.dram_tensor("sT", [128, NCtot * 128], fp8, kind="ExternalInput")
    s1_d = nc.dram_tensor("s1", [128, NCtot * 128], fp8, kind="ExternalInput")
    out_d = nc.dram_tensor("out", [W * 128, NCLS], f32, kind="ExternalOutput")

    from contextlib import ExitStack
    ctx = ExitStack()
    with tile.TileContext(nc) as tc, ctx:
        cpool = ctx.enter_context(tc.tile_pool(name="const", bufs=1))
        gpool = ctx.enter_context(tc.tile_pool(name="gather", bufs=3))
        stpool = ctx.enter_context(tc.tile_pool(name="st", bufs=3))
        spool = ctx.enter_context(tc.tile_pool(name="small", bufs=6))
        wpool = ctx.enter_context(tc.tile_pool(name="work", bufs=2))
        pspool = ctx.enter_context(tc.tile_pool(name="ps", bufs=4, space="PSUM"))
        ps2pool = ctx.enter_context(tc.tile_pool(name="ps2", bufs=2, space="PSUM"))
        dpool = ctx.enter_context(tc.tile_pool(name="dram", bufs=1, space="DRAM"))

        T1h = [dpool.tile([8 * RPC[h], T1W], fp8, name=f"T1h{h}")
               for h in range(3)]
        T2own = [dpool.tile([RPC[h], T2W], bf16, name=f"T2own{h}")
                 for h in range(3)]
        T2g = [dpool.tile([8 * RPC[h], T2W], bf16, addr_space="Shared",
                          name=f"T2g{h}") for h in range(3)]

        def load_const(dram, shape, dtype, tag):
            t = cpool.tile(shape, dtype, tag=tag)
            nc.sync.dma_start(out=t[:], in_=dram[:])
            return t

        w1_t = load_const(w1e, [F_IN, F1 + 2 * H], bf16, "c_w1")
        w2_t = load_const(w2e, [HID, F2 + 2 * H], bf16, "c_w2")
        b1_t = load_const(b1t, [128, HID], f32, "c_b1")
        b2_t = load_const(b2t, [128, NCLS], f32, "c_b2")
        ident_t = load_const(ident_d, [128, 128], f32, "c_id")
        its = [load_const(idx_d[h], list(idx_d[h].shape), i16, f"c_idx{h}")
               for h in range(3)]
        xo_t = load_const(xown, [F_IN, W * 128], bf16, "c_xown")

        AD1 = cpool.tile([128, W, H], bf16, tag="c_ad1")
        AD2 = cpool.tile([128, W, H], bf16, tag="c_ad2")

        # prologue: work buffers (pad columns zeroed once)
        t2_bufs = []
        for i in range(2):
            t2b = wpool.tile([128, T2W], bf16, tag="t2")
            nc.vector.memset(t2b[:], 0.0)
            t2_bufs.append(t2b)
        GS = 8
        # T1 row: [0:256 feats fp8 | 256:264 a_src fold as bf16 bytes | pad]
        TCOLS = T1W if sim_queues else F1 + 8
        t14_bufs = []
        for i in range(4):
            t14b = wpool.tile([128, GS, TCOLS], fp8, tag="t14", bufs=4)
            if sim_queues:
                nc.vector.memset(t14b[:, :, F1 + 8:].opt(), 0.0)
            t14_bufs.append(t14b)

        # ---------------- phase 0a: own-window a_dst folds ------------------
        for b in range(0, W, 8):
            nb = min(8, W - b)
            ps = ps2pool.tile([128, 8 * H], f32, tag="psB")
            for j in range(nb):
                nc.tensor.matmul(out=ps[:, j * H:(j + 1) * H],
                                 lhsT=xo_t[:, (b + j) * 128:(b + j + 1) * 128],
                                 rhs=w1_t[:, F1 + H:F1 + 2 * H],
                                 start=True, stop=True)
            nc.scalar.copy(AD1[:, b:b + nb, :],
                           ps[:, :nb * H].rearrange("p (w h) -> p w h", h=H))

        # ---------------- phase 0b: node table (chunk-major, global) --------
        NT = ROWS // 128                              # 400
        for t0 in range(0, NT, GS):
            n0 = t0 * 128
            xt = wpool.tile([F_IN, GS * 128], bf16, tag="xt")
            nc.sync.dma_start(out=xt[:], in_=xT[:, n0:n0 + GS * 128])
            t14 = t14_bufs[(t0 // GS) % 4]
            psF = ps2pool.tile([128, GS * H], f32, tag="adP")
            for i in range(GS):
                ps = pspool.tile([128, F1], f32, tag="psA")
                nc.tensor.matmul(out=ps[:],
                                 lhsT=xt[:, i * 128:(i + 1) * 128],
                                 rhs=w1_t[:, :F1], start=True, stop=True)
                nc.tensor.matmul(out=psF[:, i * H:(i + 1) * H],
                                 lhsT=xt[:, i * 128:(i + 1) * 128],
                                 rhs=w1_t[:, F1:F1 + H], start=True,
                                 stop=True)
                if i % 2 == 0:
                    nc.vector.tensor_copy(t14[:, i, :F1], ps[:, :F1])
                else:
                    nc.scalar.copy(t14[:, i, :F1], ps[:, :F1])
            nc.scalar.copy(
                t14[:, :, F1:F1 + 8].bitcast(bf16),
                psF[:].rearrange("p (i h) -> p i h", h=H))
            row0 = n0
            hsel = 0 if row0 < 20480 else (1 if row0 < 40960 else 2)
            off = row0 - (0, 20480, 40960)[hsel]
            out_ap = T1h[hsel][off:off + GS * 128, :TCOLS].rearrange(
                "(c p) f -> p c f", p=128)
            nc.scalar.dma_start(out=out_ap, in_=t14[:, :, :])

        # ---------------- edge phase (shared for both layers) --------------
        # Software-pipelined: at iteration gr we (a) issue gathers for group
        # gr+1, (b) run the front half (adP/wv/exp/gw) for group gr, and
        # (c) scatter+flush group gr-1 — so each engine's in-order queue
        # interleaves independent work instead of serializing the chain.
        gather_ctr = [0]

        def edge_layer(layer, Ts, FW, TW, AD, flush):
            pos = [0, 0, 0]
            NG = len(ginfo)
            rowdt = fp8 if layer == 1 else bf16

            def as_view(gt, cT):
                if layer == 1:
                    return gt[:, :, FW:FW + 8].bitcast(bf16)
                return gt[:, :, FW:FW + H]

            def issue_gather(gr):
                info = ginfo[gr]
                cH = info["cH"]
                cT = sum(cH)
                gt = gpool.tile([128, cT, TW], rowdt, tag="g")
                off = 0
                for h in range(3):
                    cnt = cH[h]
                    if cnt == 0:
                        continue
                    p = pos[h]
                    # DMASW sem lanes are assigned round-robin over SWDGE
                    # insts in SCHEDULED order and each lane locks to one
                    # queue; the scheduler may reorder gathers, so the only
                    # robust choice is a single queue (one queue still
                    # sprays across all 16 SDMA engines).
                    q = 0
                    nc.gpsimd.dma_gather(
                        gt[:, off:off + cnt, :], Ts[h][:],
                        its[h][:, p:p + 8 * cnt], cnt * 128, cnt * 128, TW,
                        single_packet=False,
                        queue_num=0 if sim_queues else q)
                    pos[h] += 8 * cnt
                    off += cnt
                ci0 = info["ci0"]
                st = stpool.tile([128, cT, 128], fp8, tag="st")
                nc.sync.dma_start(
                    out=st[:], in_=sT_d[:, ci0 * 128:(ci0 + cT) * 128])
                s1 = stpool.tile([128, cT, 128], fp8, tag="s1")
                nc.scalar.dma_start(
                    out=s1[:], in_=s1_d[:, ci0 * 128:(ci0 + cT) * 128])
                return gt, st, s1

            def front(gr, gt, st):
                info = ginfo[gr]
                cT = sum(info["cH"])
                # per-edge alpha_dst via one-hot matmul -> PSUM
                adP = ps2pool.tile([128, cT * H], f32, tag="adP")
                for (w, cl) in info["wins"]:
                    for (o, cn) in cl:
                        for ci in range(o, o + cn):
                            nc.tensor.matmul(out=adP[:, ci * H:(ci + 1) * H],
                                             lhsT=st[:, ci, :],
                                             rhs=AD[:, w, :],
                                             start=True, stop=True)
                # logits -> exp weights (bf16)
                wv = spool.tile([128, cT * H], f32, tag="wv")
                nc.vector.tensor_tensor(
                    out=wv[:].rearrange("p (c h) -> p c h", h=H),
                    in0=as_view(gt, cT),
                    in1=adP[:].rearrange("p (c h) -> p c h", h=H), op=OP.add)
                nc.vector.scalar_tensor_tensor(
                    out=wv[:], in0=wv[:], scalar=NEG, in1=wv[:],
                    op0=OP.mult, op1=OP.max)
                wvb = spool.tile([128, cT * H], bf16, tag="wvb")
                nc.scalar.activation(wvb[:], wv[:], AF.Exp)
                # weighted features: one DVE broadcast multiply; denominator
                # cols get the weights themselves via a small ACT copy
                gw = wpool.tile([128, cT, FW + H], bf16, tag="gw")
                nc.vector.tensor_tensor(
                    out=gw[:, :, 0:FW].rearrange("p c (h f) -> p c h f", h=H),
                    in0=gt[:, :, 0:FW].rearrange("p c (h f) -> p c h f", h=H),
                    in1=wvb[:].rearrange("p (c h) -> p c h", h=H).unsqueeze(3)
                        .broadcast_to([128, cT, H, FW // H]),
                    op=OP.mult)
                nc.scalar.copy(gw[:, :, FW:FW + H],
                               wvb[:].rearrange("p (c h) -> p c h", h=H))
                return gw

            def back(gr, s1, gw):
                info = ginfo[gr]
                for (w, cl) in info["wins"]:
                    cis = [ci for (o, cn) in cl for ci in range(o, o + cn)]
                    psw = pspool.tile([128, FW + H], f32, tag="psA")
                    for j, ci in enumerate(cis):
                        nc.tensor.matmul(out=psw[:], lhsT=s1[:, ci, :],
                                         rhs=gw[:, ci, :],
                                         start=(j == 0),
                                         stop=(j == len(cis) - 1))
                    flush(w, psw)

            tiles = {0: issue_gather(0)}
            prev = None
            for gr in range(NG):
                if gr + 1 < NG:
                    tiles[gr + 1] = issue_gather(gr + 1)
                gt, st, s1 = tiles.pop(gr)
                gw = front(gr, gt, st)
                if prev is not None:
                    back(prev[0], prev[1], prev[2])
                prev = (gr, s1, gw)
            back(prev[0], prev[1], prev[2])

        # ---------------- layer 1 flush: h2-table rows ---------------------
        def flush1(w, psw):
            rec = spool.tile([128, H], f32, tag="rec")
            nc.vector.tensor_scalar(out=rec[:], in0=psw[:, F1:F1 + H],
                                    scalar1=1e-16, scalar2=None, op0=OP.add)
            nc.vector.reciprocal(rec[:], rec[:])
            tmp = wpool.tile([128, H, HID], f32, tag="tmp")
            nc.vector.tensor_tensor(
                out=tmp[:], in0=psw[:, 0:F1].rearrange("p (h f) -> p h f", h=H),
                in1=rec[:].unsqueeze(2).broadcast_to([128, H, HID]),
                op=OP.mult)
            acc = wpool.tile([128, 2, HID], f32, tag="acc1")
            nc.vector.tensor_tensor(out=acc[:], in0=tmp[:, 0:2, :],
                                    in1=tmp[:, 2:4, :], op=OP.add)
            acc2 = wpool.tile([128, HID], f32, tag="acc2")
            nc.vector.tensor_tensor(out=acc2[:], in0=acc[:, 0, :],
                                    in1=acc[:, 1, :], op=OP.add)
            nc.vector.tensor_tensor(out=acc2[:], in0=acc2[:], in1=b1_t[:],
                                    op=OP.add)
            r1 = wpool.tile([128, HID], f32, tag="r1")
            nc.scalar.activation(r1[:], acc2[:], AF.Relu, scale=1.0 / H)
            psT = ps2pool.tile([HID, 128], f32, tag="psB")
            nc.tensor.transpose(out=psT[:], in_=r1[:], identity=ident_t[:])
            l1T = wpool.tile([HID, 128], bf16, tag="l1T")
            nc.scalar.copy(l1T[:], psT[:])
            ps2 = ps2pool.tile([128, F2 + 2 * H], f32, tag="psB")
            nc.tensor.matmul(out=ps2[:], lhsT=l1T[:], rhs=w2_t[:],
                             start=True, stop=True)
            t2 = t2_bufs[w % 2]
            nc.scalar.copy(t2[:, 0:F2 + H], ps2[:, 0:F2 + H])
            nc.scalar.copy(AD2[:, w, :], ps2[:, F2 + H:F2 + 2 * H])
            hsel = 0 if w < 20 else (1 if w < 40 else 2)
            off = (w - WLIM[hsel]) * 128
            nc.scalar.dma_start(out=T2own[hsel][off:off + 128, :], in_=t2[:])
            if w == W - 1:
                zt = wpool.tile([128, T2W], bf16, tag="zt")
                nc.vector.memset(zt[:], 0.0)
                nc.scalar.dma_start(out=T2own[2][1152:1280, :],
                                    in_=zt[:128, :])
            if w in (19, 39, W - 1):
                hc = (19, 39, W - 1).index(w)
                nc.gpsimd.collective_compute(
                    "AllGather", mybir.AluOpType.bypass,
                    replica_groups=[list(range(C))],
                    ins=[T2own[hc][:, :]], outs=[T2g[hc][:, :]])

        # ---------------- layer 2 flush: final output ----------------------
        def flush2(w, psw):
            rec = spool.tile([128, H], f32, tag="rec")
            nc.vector.tensor_scalar(out=rec[:], in0=psw[:, F2:F2 + H],
                                    scalar1=1e-16, scalar2=None, op0=OP.add)
            nc.vector.reciprocal(rec[:], rec[:])
            tmp = wpool.tile([128, H, NCLS], f32, tag="tmp")
            nc.vector.tensor_tensor(
                out=tmp[:], in0=psw[:, 0:F2].rearrange("p (h f) -> p h f", h=H),
                in1=rec[:].unsqueeze(2).broadcast_to([128, H, NCLS]),
                op=OP.mult)
            acc = wpool.tile([128, 2, NCLS], f32, tag="acc1")
            nc.vector.tensor_tensor(out=acc[:], in0=tmp[:, 0:2, :],
                                    in1=tmp[:, 2:4, :], op=OP.add)
            o2 = wpool.tile([128, NCLS], f32, tag="o2")
            nc.vector.tensor_tensor(out=o2[:], in0=acc[:, 0, :],
                                    in1=acc[:, 1, :], op=OP.add)
            nc.vector.scalar_tensor_tensor(
                out=o2[:], in0=o2[:], scalar=1.0 / H, in1=b2_t[:],
                op0=OP.mult, op1=OP.add)
            nc.scalar.dma_start(out=out_d[w * 128:(w + 1) * 128, :],
                                in_=o2[:])

        edge_layer(1, T1h, F1, T1W, AD1, flush1)
        edge_layer(2, T2g, F2, T2W, AD2, flush2)

    nc.compile()
    return nc


# ---------------------------------------------------------------------------
# host driver
# ---------------------------------------------------------------------------

def _install_ntff_hook():
    try:
        from trn_agent_boot.trn_boot import _ntff_profile_via_ctypes
        hook = _ntff_profile_via_ctypes("/opt/axon/libaxon_pjrt.so")
        m = types.ModuleType("antenv.axon_hooks")
        m.get_axon_ntff_profile_hook = lambda: hook
        m.set_axon_ntff_profile_hook = lambda h: None
        sys.modules["antenv.axon_hooks"] = m
    except Exception:
        pass


def make_inputs(x, edge_index, W1, a_src1, a_dst1, b1, W2, a_src2, a_dst2, b2,
                cfg):
    N, C, H = cfg["N"], cfg["CORES"], cfg["HEADS"]
    loops = np.arange(N, dtype=np.int64)
    src = np.concatenate([np.asarray(edge_index[0], np.int64) % N, loops])
    dst = np.concatenate([np.asarray(edge_index[1], np.int64) % N, loops])
    common, per_core = build_schedules(src, dst, cfg)
    W1e, W2e = fold_params(np.asarray(W1, np.float32), np.asarray(a_src1),
                           np.asarray(a_dst1), np.asarray(W2),
                           np.asarray(a_src2), np.asarray(a_dst2), cfg)
    ident = np.eye(128, dtype=np.float32)
    b1t = np.broadcast_to(np.asarray(b1, np.float32) * H, (128, cfg["HID"]))
    b2t = np.broadcast_to(np.asarray(b2, np.float32) * H, (128, cfg["NCLS"]))
    x = np.asarray(x, np.float32)

    # third-major global xT (identical on all cores)
    row = np.arange(ROWS)
    third = np.where(row < 20480, 0, np.where(row < 40960, 1, 2))
    rel = row - np.array(RBASE)[third]
    rpc = np.array(RPC)[third]
    cn = rel // rpc
    r = rel % rpc + np.array((0, 2560, 5120))[third]
    valid = r < NPC
    node = cn * NPC + np.minimum(r, NPC - 1)
    xcols = np.zeros((ROWS, cfg["F_IN"]), np.float32)
    xcols[valid] = x[node[valid]]
    xT_g = np.ascontiguousarray(xcols.T).astype(BF16)

    in_maps = []
    for c in range(C):
        m = dict(per_core[c])
        m["xT"] = xT_g
        own = np.zeros((W * 128, cfg["F_IN"]), np.float32)
        own[:NPC] = x[c * NPC:(c + 1) * NPC]
        m["xown"] = np.ascontiguousarray(own.T).astype(BF16)
        m["w1e"] = W1e.astype(BF16)
        m["w2e"] = W2e.astype(BF16)
        m["b1t"] = np.ascontiguousarray(b1t)
        m["b2t"] = np.ascontiguousarray(b2t)
        m["ident"] = ident
        in_maps.append(m)
    return common, in_maps


def kernel(x, edge_index, W1, a_src1, a_dst1, b1, W2, a_src2, a_dst2, b2,
           cfg=None, trace=False, sim=False):
    cfg = cfg or FULL_CFG
    _install_ntff_hook()
    from concourse.bass_utils import run_bass_kernel_spmd

    common, in_maps = make_inputs(x, edge_index, W1, a_src1, a_dst1, b1,
                                  W2, a_src2, a_dst2, b2, cfg)
    nc = build_program(cfg, common, sim_queues=sim)
    C = cfg["CORES"]

    if sim:
        import concourse.bass_interp as bass_interp
        # bf16 folds bitcast into fp8 tables false-positive the fp8 NaN check
        s = bass_interp.MultiCoreSim(nc, C, require_nnan=False,
                                     require_finite=False)
        for c in range(C):
            for kk, v in in_maps[c].items():
                s.cores[c].tensor(kk)[:] = v
        s.simulate()
        outs = [np.array(s.cores[c].tensor("out")) for c in range(C)]
        kernel.last_exec_ns = None
    else:
        res = run_bass_kernel_spmd(nc, in_maps, list(range(C)), trace=trace)
        outs = [res.results[c]["out"] for c in range(C)]
        kernel.last_exec_ns = res.exec_time_ns
    return np.concatenate([o[:NPC] for o in outs], axis=0)
